# revision 1
# baseline (speedup 1.0000x reference)
"""Locally-connected layer (unshared 3x3 conv, torch-unfold semantics) on 8 trn2 cores.

out[b,o,y,x] = sum_{c,i,j} weight[o, c*9+i*3+j, y*32+x] * xpad[b, c, y+i, x+j] + bias[o, l]

Sharding: spatial over L — core r owns image rows [4r, 4r+4) (128 pixels).
Each core receives:
  x slab  (B, C, 6, 34)  : host-zero-padded rows 4r-1..4r+4, width padded to 34
  w slab  (O, 576, 128)  : weight[:, :, 128r : 128r+128]
Output per core: (B, O, 128); host concatenates along L.

On-chip: the padded input slab is stored as XP[c partitions, (b, y', x') free];
the im2col is done purely with access-pattern offsets into this slab (no data
movement).  Partitions 64:128 hold a copy of the slab shifted by one column
(+1 elem), so kernel columns j=0 and j=1 fuse into single K=128 matmuls:
  per pixel: 3 fused matmuls (i x j in {0,1}) + 3 singles (i, j=2) as 64-row
  matmuls on partitions 0:64 -> 6 fp32 matmuls accumulated in PSUM.
(All matmuls start at partition 0: fp32 matmul groups that mix 64-row blocks
at base 0 and base 64 crash TRN2 — bisected on HW.)
"""

import numpy as np

B, C, O, H, W, KS = 64, 64, 64, 32, 32, 3
L = H * W
NCORES = 8
RPC = H // NCORES            # image rows per core = 4
LC = RPC * W                 # pixels per core = 128
HALO = RPC + 2               # 6 slab rows
WP = W + 2                   # padded width 34
BST = HALO * WP              # per-b free stride in XP = 204

_CACHE = {}


def _build_nc():
    import concourse.bass as bass
    import concourse.bacc as bacc
    import concourse.tile as tile
    from concourse import mybir

    f32 = mybir.dt.float32
    nc = bacc.Bacc(
        "TRN2", target_bir_lowering=False, debug=False, num_devices=NCORES
    )
    x_d = nc.dram_tensor("x", [B, C, HALO, WP], f32, kind="ExternalInput")
    w_d = nc.dram_tensor("w", [O, C * KS * KS, LC], f32, kind="ExternalInput")
    o_d = nc.dram_tensor("out", [B, O, LC], f32, kind="ExternalOutput")

    with tile.TileContext(nc) as tc:
        with (
            tc.tile_pool(name="xp", bufs=1) as xpool,
            tc.tile_pool(name="w0", bufs=2) as wp0,
            tc.tile_pool(name="w1", bufs=2) as wp1,
            tc.tile_pool(name="w2", bufs=2) as wp2,
            tc.tile_pool(name="s0", bufs=2) as sp0,
            tc.tile_pool(name="s1", bufs=2) as sp1,
            tc.tile_pool(name="s2", bufs=2) as sp2,
            tc.tile_pool(name="orow", bufs=2) as opool,
            tc.tile_pool(name="ps", bufs=8, space=bass.MemorySpace.PSUM) as pspool,
        ):
            # ---- input slab: XP[c, (b, y', x')] on partitions 0:64, plus a
            # +1-column-shifted copy on partitions 64:128 so (j=0, j=1)
            # kernel-column pairs share one K=128 access pattern.
            xp = xpool.tile([128, B * BST], f32)
            xp3 = xp[:].rearrange("p (b f) -> p b f", f=BST)
            xin = x_d[:].rearrange("b c h w -> c b (h w)")
            nc.sync.dma_start(xp3[0:64], xin)
            nc.sync.dma_start(xp3[64:128, :, 0 : BST - 1], xin[:, :, 1:])

            # weight viewed as [i, j, c, o, l]  (k = c*9 + i*3 + j)
            w_v = w_d[:].rearrange("o (c i j) l -> i j c o l", i=KS, j=KS)

            wpools = [wp0, wp1, wp2]
            spools = [sp0, sp1, sp2]
            for y in range(RPC):
                lsl = slice(y * W, (y + 1) * W)
                wts, sts = [], []
                for i in range(KS):
                    # (i, j=0) on partitions 0:64, (i, j=1) on 64:128
                    wt = wpools[i].tile([128, O, W], f32)
                    nc.sync.dma_start(wt[0:64], w_v[i, 0, :, :, lsl])
                    nc.sync.dma_start(wt[64:128], w_v[i, 1, :, :, lsl])
                    wts.append(wt)
                    # (i, j=2) on partitions 0:64
                    st = spools[i].tile([64, O, W], f32)
                    nc.sync.dma_start(st[:], w_v[i, 2, :, :, lsl])
                    sts.append(st)

                orow = opool.tile([B, O, W], f32)
                for x in range(W):
                    ps = pspool.tile([B, O], f32)

                    def off(i, j, _y=y, _x=x):
                        return (_y + i) * WP + _x + j

                    # fused (j=0 lower, j=1 upper via shifted copy), K=128
                    for i in range(KS):
                        nc.tensor.matmul(
                            ps[:], xp3[0:128, :, off(i, 0)], wts[i][:, :, x],
                            start=(i == 0), stop=False,
                        )
                    # singles (i, j=2), K=64 at base partition 0
                    for i in range(KS):
                        nc.tensor.matmul(
                            ps[:], xp3[0:64, :, off(i, 2)], sts[i][:, :, x],
                            start=False, stop=(i == KS - 1),
                        )
                    nc.vector.tensor_copy(orow[:, :, x], ps[:])
                nc.sync.dma_start(o_d[:, :, lsl], orow[:])
    nc.compile()
    return nc


def _get_nc():
    if "nc" not in _CACHE:
        _CACHE["nc"] = _build_nc()
    return _CACHE["nc"]


def _shard_inputs(x, weight):
    xpad = np.pad(x, ((0, 0), (0, 0), (1, 1), (1, 1)))
    in_maps = []
    for r in range(NCORES):
        xs = np.ascontiguousarray(xpad[:, :, RPC * r : RPC * r + HALO, :])
        ws = np.ascontiguousarray(weight[:, :, LC * r : LC * (r + 1)])
        in_maps.append({"x": xs, "w": ws})
    return in_maps


def kernel(x, weight, bias, _trace=False, _trace_kwargs=None):
    from concourse.bass_utils import run_bass_kernel_spmd

    x = np.ascontiguousarray(np.asarray(x, dtype=np.float32))
    weight = np.asarray(weight, dtype=np.float32)
    bias = np.asarray(bias, dtype=np.float32)

    nc = _get_nc()
    in_maps = _shard_inputs(x, weight)
    res = run_bass_kernel_spmd(
        nc, in_maps, list(range(NCORES)),
        trace=_trace, **(_trace_kwargs or {}),
    )
    out = np.concatenate(
        [res.results[r]["out"] for r in range(NCORES)], axis=2
    ).reshape(B, O, H, W)
    if np.any(bias):
        # bias is all-zeros in this problem's setup; fold on host if not.
        out = out + bias.reshape(1, O, H, W)
    if _trace:
        _CACHE["last_result"] = res
    return np.ascontiguousarray(out.astype(np.float32))



# revision 2
# speedup vs baseline: 4.1505x; 4.1505x over previous
"""Locally-connected layer (unshared 3x3 conv, torch-unfold semantics) on 8 trn2 cores.

out[b,o,y,x] = sum_{c,i,j} weight[o, c*9+i*3+j, y*32+x] * xpad[b, c, y+i, x+j]

Sharding: spatial over L — core r owns image rows [4r, 4r+4) (128 pixels).

v2 design (bf16, pixel-pair col-tiling, 5 contraction chunks):
  * Everything is bf16 on the wire (tolerance 2e-2; bf16 contraction error
    ~0.5%).  PSUM accumulates fp32.
  * The 576-long contraction (c,i,j) is reordered into 5 chunks:
      q0..q2: K=128 = [c x (i=q,j=0) | c x (i=q,j=1)]   (rhs = T1 slab pair)
      q3:     K=128 = [c x (0,2)     | c x (1,2)    ]   (rhs = T2 slab pair)
      q4:     K=64  =  c x (2,2)                        (rhs = T1 lower)
    T1 = [slab | slab shifted +1 col], T2 = [slab | slab shifted +1 row]
    (T2 built on-chip via SBUF->SBUF DMA from T1's lower half).
  * Weights are the matmul stationary operand, host-packed bf16 in the exact
    SBUF layout so every weight DMA is a large fully-contiguous read:
    one [K, 128] stationary per chunk covers BOTH pixels of a pair
    (cols 0:64 = even pixel's o, 64:128 = odd pixel's o); the two matmuls
    write PSUM partitions 0:64 / 64:128 (col-tiled, run concurrently).
  * Output stays bf16 in a [psum-partition, pair, b] DRAM layout (contiguous
    DMA); the host transposes to (B, O, H, W) fp32.
"""

import numpy as np
import ml_dtypes

BF16 = ml_dtypes.bfloat16

B, C, O, H, W, KS = 64, 64, 64, 32, 32, 3
L = H * W
NCORES = 8
RPC = H // NCORES            # image rows per core = 4
LC = RPC * W                 # pixels per core = 128
NP = LC // 2                 # pixel pairs per core = 64
HALO = RPC + 2               # 6 slab rows
WP = W + 2                   # padded width 34
BST = HALO * WP              # per-b free stride in the slab = 204
PG = 8                       # pairs per weight DMA group
NG = NP // PG                # weight groups = 8

_CACHE = {}


def _build_nc():
    import concourse.bass as bass
    import concourse.bacc as bacc
    import concourse.tile as tile
    from concourse import mybir

    f32 = mybir.dt.float32
    bf16 = mybir.dt.bfloat16
    nc = bacc.Bacc(
        "TRN2", target_bir_lowering=False, debug=False, num_devices=NCORES
    )
    x_d = nc.dram_tensor("x", [128, B * BST], bf16, kind="ExternalInput")
    wq_d = nc.dram_tensor("wq", [NG, 128, PG, 4, 128], bf16, kind="ExternalInput")
    ws_d = nc.dram_tensor("ws", [NG, 64, PG, 128], bf16, kind="ExternalInput")
    o_d = nc.dram_tensor("out", [128, NP, B], bf16, kind="ExternalOutput")

    with tile.TileContext(nc) as tc:
        with (
            tc.tile_pool(name="x1", bufs=1) as x1pool,
            tc.tile_pool(name="x2", bufs=1) as x2pool,
            tc.tile_pool(name="wq", bufs=3) as wpool,
            tc.tile_pool(name="ws", bufs=3) as spool,
            tc.tile_pool(name="orow", bufs=2) as opool,
            tc.tile_pool(name="ps", bufs=8, space=bass.MemorySpace.PSUM) as pspool,
        ):
            # T1 = [slab | slab+1col] straight from HBM (host pre-built).
            t1 = x1pool.tile([128, B * BST], bf16)
            t13 = t1[:].rearrange("p (b f) -> p b f", f=BST)
            nc.sync.dma_start(t1[:], x_d[:])

            # T2 = [slab | slab+1row], both halves copied on-chip from T1
            # lower (SBUF->SBUF; no HBM traffic).
            t2 = x2pool.tile([128, B * BST], bf16)
            t23 = t2[:].rearrange("p (b f) -> p b f", f=BST)
            nc.sync.dma_start(t23[0:64], t13[0:64])
            nc.sync.dma_start(
                t23[64:128, :, 0 : BST - WP], t13[0:64, :, WP:BST]
            )

            for g in range(NG):
                wt = wpool.tile([128, PG, 4, 128], bf16)
                st = spool.tile([64, PG, 128], bf16)
                nc.sync.dma_start(wt[:], wq_d[g])
                nc.sync.dma_start(st[:], ws_d[g])
                orow = opool.tile([128, PG, B], bf16)
                for tt in range(PG):
                    t = g * PG + tt
                    y, x0 = (2 * t) // W, (2 * t) % W
                    ps = pspool.tile([128, B], f32)
                    for q in range(4):
                        if q < 3:
                            off = (y + q) * WP + x0
                            rv = t13
                        else:
                            off = y * WP + x0 + 2
                            rv = t23
                        nc.tensor.matmul(
                            ps[0:64], wt[:, tt, q, 0:64], rv[:, :, off],
                            start=(q == 0), stop=False,
                        )
                        nc.tensor.matmul(
                            ps[64:128], wt[:, tt, q, 64:128], rv[:, :, off + 1],
                            start=(q == 0), stop=False,
                        )
                    offs = (y + 2) * WP + x0 + 2
                    nc.tensor.matmul(
                        ps[0:64], st[:, tt, 0:64], t13[0:64, :, offs],
                        start=False, stop=True,
                    )
                    nc.tensor.matmul(
                        ps[64:128], st[:, tt, 64:128], t13[0:64, :, offs + 1],
                        start=False, stop=True,
                    )
                    nc.vector.tensor_copy(orow[:, tt, :], ps[:])
                nc.sync.dma_start(o_d[:, g * PG : (g + 1) * PG, :], orow[:])
    nc.compile()
    return nc


def _get_nc():
    if "nc" not in _CACHE:
        _CACHE["nc"] = _build_nc()
    return _CACHE["nc"]


def _pack_x(x):
    """Per core: [128, B*BST] bf16 = [slab | slab shifted +1 col]."""
    xpad = np.pad(x, ((0, 0), (0, 0), (1, 1), (1, 1)))
    xpad = np.ascontiguousarray(xpad.transpose(1, 0, 2, 3))  # [C, B, 34, 34]
    outs = []
    for r in range(NCORES):
        slab = xpad[:, :, RPC * r : RPC * r + HALO, :].reshape(C, B, BST)
        up = np.zeros_like(slab)
        up[:, :, : BST - 1] = slab[:, :, 1:]
        t1 = np.concatenate([slab, up], axis=0).astype(BF16)
        outs.append(np.ascontiguousarray(t1.reshape(128, B * BST)))
    return outs


def _pack_w(weight):
    """Chunked-contraction weight blobs, already in SBUF layout.

    wq: [core, NG, p=(half, c), tt, q, m=(e, o)]  (4 K>=128 chunks)
    ws: [core, NG, c, tt, m=(e, o)]               (the K=64 (2,2) single)
    """
    w5 = weight.reshape(O, C, KS, KS, L)
    low = np.stack(
        [w5[:, :, 0, 0], w5[:, :, 1, 0], w5[:, :, 2, 0], w5[:, :, 0, 2]], axis=0
    )
    up = np.stack(
        [w5[:, :, 0, 1], w5[:, :, 1, 1], w5[:, :, 2, 1], w5[:, :, 1, 2]], axis=0
    )
    wq = np.stack([low, up], axis=1)          # [q, half, O, C, L]
    wq = wq.reshape(4, 2, O, C, NCORES, NG, PG, 2)
    # -> [core, g, half, c, tt, q, e, o]
    wq = wq.transpose(4, 5, 1, 3, 6, 0, 7, 2)
    wq = np.ascontiguousarray(wq, dtype=BF16).reshape(NCORES, NG, 128, PG, 4, 128)

    ws = w5[:, :, 2, 2].reshape(O, C, NCORES, NG, PG, 2)
    ws = ws.transpose(2, 3, 1, 4, 5, 0)       # [core, g, c, tt, e, o]
    ws = np.ascontiguousarray(ws, dtype=BF16).reshape(NCORES, NG, 64, PG, 128)
    return wq, ws


def kernel(x, weight, bias, _trace=False, _trace_kwargs=None):
    from concourse.bass_utils import run_bass_kernel_spmd

    x = np.asarray(x, dtype=np.float32)
    weight = np.asarray(weight, dtype=np.float32)
    bias = np.asarray(bias, dtype=np.float32)

    nc = _get_nc()
    xs = _pack_x(x)
    wq, ws = _pack_w(weight)
    in_maps = [
        {"x": xs[r], "wq": wq[r], "ws": ws[r]} for r in range(NCORES)
    ]
    res = run_bass_kernel_spmd(
        nc, in_maps, list(range(NCORES)),
        trace=_trace, **(_trace_kwargs or {}),
    )
    # out[r]: [p=(e,o), t, b] bf16 -> [b, o, l=128r+2t+e]
    parts = []
    for r in range(NCORES):
        arr = res.results[r]["out"].astype(np.float32)
        arr = arr.reshape(2, O, NP, B).transpose(3, 1, 2, 0)  # [b, o, t, e]
        parts.append(arr.reshape(B, O, LC))
    out = np.concatenate(parts, axis=2).reshape(B, O, H, W)
    if np.any(bias):
        out = out + bias.reshape(1, O, H, W)
    if _trace:
        _CACHE["last_result"] = res
    return np.ascontiguousarray(out.astype(np.float32))


# revision 7
# speedup vs baseline: 4.6100x; 1.1107x over previous
"""Locally-connected layer (unshared 3x3 conv, torch-unfold semantics) on 8 trn2 cores.

out[b,o,y,x] = sum_{c,i,j} weight[o, c*9+i*3+j, y*32+x] * xpad[b, c, y+i, x+j]

Sharding: spatial over L — core r owns image rows [4r, 4r+4) (128 pixels).

v3 design (bf16, weights-stationary, N=128 moving, single slab tile):
  * Everything bf16 on the wire (tolerance 2e-2; bf16 error ~1%). PSUM fp32.
  * SBUF slab T1 [128, B*204] = [slab | slab shifted +1 col] (host-built,
    one contiguous HBM DMA).  All im2col is pure access-pattern offsets.
  * The 576-long contraction is reordered into chunks whose stationary is a
    [K, 128] host-packed weight block covering BOTH pixels of a pair
    (cols m = 64*e + o: pixel parity e, channel o).  Moving operand is
    x [K, N=128=(b, pix)] read as t13[:, :, off:off+2].  PSUM [128, (b,pix)]:
    only the e==pix halves are read out, the rest is discarded.
      q0..q2: K=128  rows [c x (i=q,j=0) | c x (i=q,j=1)]  rhs t13[0:128] @ off
      s0..s2: K=64   rows  c x (s,2)  (ws blob, rows 0:64) rhs t13[0:64]
    All matmuls sit at row base 0 (mixed-base accumulation groups crash TRN2).
  * 6 matmuls / 6 ldweights per pixel pair (384 per core), all with 128-col
    stationaries (FWL-eligible) and N=128 moving.  PSUM readout alternates
    vector/scalar by pair so the two engines touch different PSUM banks.
  * Output bf16 in [psum-partition, pair, b] DRAM layout (contiguous DMA);
    host transposes to (B, O, H, W) fp32.
"""

import numpy as np
import ml_dtypes

BF16 = ml_dtypes.bfloat16

B, C, O, H, W, KS = 64, 64, 64, 32, 32, 3
L = H * W
NCORES = 8
RPC = H // NCORES            # image rows per core = 4
LC = RPC * W                 # pixels per core = 128
NP = LC // 2                 # pixel pairs per core = 64
HALO = RPC + 2               # 6 slab rows
WP = W + 2                   # padded width 34
BST = HALO * WP              # per-b free stride in the slab = 204
PG = 8                       # pairs per weight DMA group
NG = NP // PG                # weight groups = 8

_CACHE = {}


def _build_nc():
    import concourse.bass as bass
    import concourse.bacc as bacc
    import concourse.tile as tile
    from concourse import mybir

    f32 = mybir.dt.float32
    bf16 = mybir.dt.bfloat16
    nc = bacc.Bacc(
        "TRN2", target_bir_lowering=False, debug=False, num_devices=NCORES
    )
    x_d = nc.dram_tensor("x", [128, B * BST], bf16, kind="ExternalInput")
    wq_d = nc.dram_tensor("wq", [NG, 128, PG, 3, 128], bf16, kind="ExternalInput")
    ws_d = nc.dram_tensor("ws", [NG, 64, PG, 3, 128], bf16, kind="ExternalInput")
    o_d = nc.dram_tensor("out", [128, NP, B], bf16, kind="ExternalOutput")

    with tile.TileContext(nc) as tc:
        with (
            tc.tile_pool(name="x1", bufs=1) as x1pool,
            tc.tile_pool(name="wq", bufs=4) as wpool,
            tc.tile_pool(name="ws", bufs=4) as spool,
            tc.tile_pool(name="orow", bufs=2) as opool,
            tc.tile_pool(name="ps", bufs=8, space=bass.MemorySpace.PSUM) as pspool,
        ):
            t1 = x1pool.tile([128, B * BST], bf16)
            t13 = t1[:].rearrange("p (b f) -> p b f", f=BST)
            # split into two DMAs so both halves stream on parallel queues
            nc.sync.dma_start(t1[0:64], x_d[0:64])
            nc.sync.dma_start(t1[64:128], x_d[64:128])

            for g in range(NG):
                wt = wpool.tile([128, PG, 3, 128], bf16)
                st = spool.tile([64, PG, 3, 128], bf16)
                nc.sync.dma_start(wt[:], wq_d[g])
                nc.sync.dma_start(st[:], ws_d[g])
                orow = opool.tile([128, PG, B], bf16)
                for tt in range(PG):
                    t = g * PG + tt
                    y, x0 = (2 * t) // W, (2 * t) % W
                    ps = pspool.tile([128, B, 2], f32)
                    for q in range(3):
                        off = (y + q) * WP + x0
                        nc.tensor.matmul(
                            ps[:], wt[:, tt, q, :], t13[:, :, off : off + 2],
                            start=(q == 0), stop=False,
                        )
                    for s in range(3):
                        offs = (y + s) * WP + x0 + 2
                        nc.tensor.matmul(
                            ps[:], st[:, tt, s, :], t13[0:64, :, offs : offs + 2],
                            start=False, stop=(s == 2),
                        )
                    eng = nc.vector if tt % 2 == 0 else nc.scalar
                    if tt % 2 == 0:
                        nc.vector.tensor_copy(orow[0:64, tt, :], ps[0:64, :, 0])
                        nc.vector.tensor_copy(orow[64:128, tt, :], ps[64:128, :, 1])
                    else:
                        nc.scalar.copy(orow[0:64, tt, :], ps[0:64, :, 0])
                        nc.scalar.copy(orow[64:128, tt, :], ps[64:128, :, 1])
                nc.sync.dma_start(o_d[:, g * PG : (g + 1) * PG, :], orow[:])
    nc.compile()
    return nc


def _get_nc():
    if "nc" not in _CACHE:
        _CACHE["nc"] = _build_nc()
    return _CACHE["nc"]


def _pack_x(x):
    """Per core: [128, B*BST] bf16 = [slab | slab shifted +1 col]."""
    xpad = np.pad(x, ((0, 0), (0, 0), (1, 1), (1, 1)))
    xpad = np.ascontiguousarray(xpad.transpose(1, 0, 2, 3))  # [C, B, 34, 34]
    outs = []
    for r in range(NCORES):
        slab = xpad[:, :, RPC * r : RPC * r + HALO, :].reshape(C, B, BST)
        up = np.zeros_like(slab)
        up[:, :, : BST - 1] = slab[:, :, 1:]
        t1 = np.concatenate([slab, up], axis=0).astype(BF16)
        outs.append(np.ascontiguousarray(t1.reshape(128, B * BST)))
    return outs


def _pack_w(weight):
    """Chunked-contraction weight blobs, already in SBUF layout.

    wq: [core, NG, p=(j, c), tt, q, m=(e, o)]   (pair chunks, shifts (q, j))
    ws: [core, NG, c, tt, s, m=(e, o)]          (singles, shifts (s, 2))
    """
    w5 = weight.reshape(O, C, KS, KS, L)
    low = np.stack([w5[:, :, 0, 0], w5[:, :, 1, 0], w5[:, :, 2, 0]], axis=0)
    up = np.stack([w5[:, :, 0, 1], w5[:, :, 1, 1], w5[:, :, 2, 1]], axis=0)
    wq = np.stack([low, up], axis=1)          # [q, j, O, C, L]
    wq = wq.reshape(3, 2, O, C, NCORES, NG, PG, 2)
    # -> [core, g, j, c, tt, q, e, o]
    wq = wq.transpose(4, 5, 1, 3, 6, 0, 7, 2)
    wq = np.ascontiguousarray(wq, dtype=BF16).reshape(NCORES, NG, 128, PG, 3, 128)

    ws = np.stack([w5[:, :, 0, 2], w5[:, :, 1, 2], w5[:, :, 2, 2]], axis=0)
    ws = ws.reshape(3, O, C, NCORES, NG, PG, 2)
    ws = ws.transpose(3, 4, 2, 5, 0, 6, 1)    # [core, g, c, tt, s, e, o]
    ws = np.ascontiguousarray(ws, dtype=BF16).reshape(NCORES, NG, 64, PG, 3, 128)
    return wq, ws


def kernel(x, weight, bias, _trace=False, _trace_kwargs=None):
    from concourse.bass_utils import run_bass_kernel_spmd

    x = np.asarray(x, dtype=np.float32)
    weight = np.asarray(weight, dtype=np.float32)
    bias = np.asarray(bias, dtype=np.float32)

    nc = _get_nc()
    xs = _pack_x(x)
    wq, ws = _pack_w(weight)
    in_maps = [
        {"x": xs[r], "wq": wq[r], "ws": ws[r]} for r in range(NCORES)
    ]
    res = run_bass_kernel_spmd(
        nc, in_maps, list(range(NCORES)),
        trace=_trace, **(_trace_kwargs or {}),
    )
    # out[r]: [p=(e,o), t, b] bf16 -> [b, o, l=128r+2t+e]
    parts = []
    for r in range(NCORES):
        arr = res.results[r]["out"].astype(np.float32)
        arr = arr.reshape(2, O, NP, B).transpose(3, 1, 2, 0)  # [b, o, t, e]
        parts.append(arr.reshape(B, O, LC))
    out = np.concatenate(parts, axis=2).reshape(B, O, H, W)
    if np.any(bias):
        out = out + bias.reshape(1, O, H, W)
    if _trace:
        _CACHE["last_result"] = res
    return np.ascontiguousarray(out.astype(np.float32))
